# revision 27
# baseline (speedup 1.0000x reference)
"""Multi-head attention (B=8, H=8, S=1024, d=128) on 8 TRN2 NeuronCores.

Strategy
--------
- Data-parallel over batch: core i handles batch i (8 cores, B=8).
- Host-side prep (layout only): per batch, compact keys/values to the
  seq_mask-selected rows (zero-padded to a multiple of 128 -> kt_tiles
  k-tiles), pre-transpose Q and compacted K so the contraction dim (d)
  lands on SBUF partitions, and cast matmul operands to fp16 (10-bit
  mantissa, same precision class as the PE's TF32 path, but 2-byte
  weight loads that background-prefetch). An indicator matrix
  ind[k, 32] (1 for real keys) rides along for the softmax denominator.
- Device math per (head h, k-tile kt), all matmuls as column-tiled M=64
  (or M=32) pairs on disjoint PE column groups + disjoint PSUM banks so
  the two streams run concurrently:
    logitsT[k, q]  = K^T[:, kt].T @ Q^T           (PE, contraction d)
    W^T[k, q]      = exp(logitsT * d^-0.5)        (ACT, PSUM -> SBUF fp16)
    outT[d, q]    += V[kt].T   @ W^T              (PE, accum over kt)
    den[q]        += ind[kt].T @ W^T              (PE, accum over kt)
  then copy outT/den to SBUF (DVE) and DMA out; the division happens on
  the host (DVE reciprocal is microcoded and slow).
  The learned scalar bias b cancels in softmax (shift invariance) and
  the -1e30 masking is equivalent to dropping masked keys (exp -> 0),
  which the compaction does exactly.
- Host-side unshard: out[b] = outT.T / den (plus uniform-average
  fallback for a fully-masked batch, where reference degenerates to a
  uniform softmax).
"""
from contextlib import ExitStack

import numpy as np

import concourse.bacc as bacc
import concourse.bass as bass
import concourse.mybir as mybir
import concourse.tile as tile
from concourse.bass_utils import run_bass_kernel_spmd

F32 = mybir.dt.float32
F32R = mybir.dt.float32r
F16 = mybir.dt.float16

B, S, D, H = 8, 1024, 1024, 8
DH = D // H              # 128, head dim = one partition tile
SCALE = float(DH) ** -0.5
NQC = S // 512           # q chunks of 512 for AV/normalize

_NC_CACHE: dict[tuple, object] = {}

# build options (overridable for profiling experiments)
OPTS: dict = {}


def _build(kt_tiles: int, opts: dict | None = None):
    """Build + compile the per-core kernel for `kt_tiles` 128-wide key tiles."""
    opts = opts or {}
    pl_bufs = opts.get("pl_bufs", 2)
    po_bufs = opts.get("po_bufs", 1)
    copy_eng = opts.get("copy_eng", "vector")   # engine for PSUM->SBUF copies
    store_eng = opts.get("store_eng", "sync")  # engine issuing output DMAs
    KP = kt_tiles * 128
    nc = bacc.Bacc("TRN2", target_bir_lowering=False, debug=False)

    q_t = nc.dram_tensor("q_t", [D, S], F16, kind="ExternalInput")
    k_t = nc.dram_tensor("k_t", [D, KP], F16, kind="ExternalInput")
    v_c = nc.dram_tensor("v_c", [KP, D], F16, kind="ExternalInput")
    ind = nc.dram_tensor("ind", [KP, 32], F16, kind="ExternalInput")
    out_t = nc.dram_tensor("out_t", [D, S], F32, kind="ExternalOutput")
    den_t = nc.dram_tensor("den_t", [H, 33, S], F32, kind="ExternalOutput")

    with tile.TileContext(nc) as tc, ExitStack() as ctx:
        sb_in = ctx.enter_context(tc.tile_pool(name="sb_in", bufs=3))
        sb_ind = ctx.enter_context(tc.tile_pool(name="sb_ind", bufs=1))
        sb_w = ctx.enter_context(tc.tile_pool(name="sb_w", bufs=6))
        sb_out = ctx.enter_context(tc.tile_pool(name="sb_out", bufs=4))
        ps_l = ctx.enter_context(
            tc.tile_pool(name="ps_l", bufs=pl_bufs, space="PSUM"))
        ps_o = ctx.enter_context(
            tc.tile_pool(name="ps_o", bufs=po_bufs, space="PSUM"))
        ps_d = ctx.enter_context(tc.tile_pool(name="ps_d", bufs=1, space="PSUM"))

        copy = nc.vector if copy_eng == "vector" else nc.scalar
        store = {"gpsimd": nc.gpsimd, "scalar": nc.scalar, "sync": nc.sync}[store_eng]

        def copy_op(dst, src):
            if copy_eng == "vector":
                nc.vector.tensor_copy(dst, src)
            else:
                nc.scalar.copy(dst, src)

        ind_sb = None

        for h in range(H):
            hs = h * DH
            # first chunks first: the kernel's first matmul needs
            # kth[:, :128] and qth[:, :512] only
            kth = sb_in.tile([128, KP], F16, tag="kth")
            nc.sync.dma_start(kth[:, 0:128], k_t.ap()[hs:hs + DH, 0:128])
            qth = sb_in.tile([128, S], F16, tag="qth")
            nc.sync.dma_start(qth[:, 0:512], q_t.ap()[hs:hs + DH, 0:512])
            if KP > 128:
                nc.sync.dma_start(kth[:, 128:], k_t.ap()[hs:hs + DH, 128:])
            nc.sync.dma_start(qth[:, 512:], q_t.ap()[hs:hs + DH, 512:])
            if ind_sb is None:
                # indicator tiles: [128(k), 32] per k-tile, concatenated on
                # the free dim. 32 columns keeps their LDWEIGHTS cheap.
                ind_sb = sb_ind.tile([128, kt_tiles * 32], F16)
                nc.sync.dma_start(
                    ind_sb[:].rearrange("p (t c) -> p t c", c=32),
                    ind.ap().rearrange("(t p) c -> p t c", p=128),
                )
            # V for this head: [128(k), 128(d)] tiles concatenated on free dim
            vh = sb_in.tile([128, KP], F16, tag="vh")
            nc.sync.dma_start(
                vh[:].rearrange("p (t c) -> p t c", c=DH),
                v_c.ap()[:, hs:hs + DH].rearrange("(t p) c -> p t c", p=128),
            )

            po = ps_o.tile([128, S], F32, tag="po")    # outT accum [d, q]
            # denominator: [0:32, 0:512] = q-chunk 0, [32:64, 512:] = q-chunk 1
            pd = ps_d.tile([64, S], F32, tag="pd")

            # Column-tiled matmul pairs: two M=64 matmuls on disjoint PE
            # column groups AND disjoint PSUM banks stream concurrently.
            # Pair diagonally across (column-half, q-chunk) so banks differ.
            s0, s1 = slice(0, 512), slice(512, 1024)
            wts = []

            def emit_qk(kt):
                pl = ps_l.tile([128, S], F32, tag="pl", name=f"pl_{h}_{kt}")
                ks = kt * 128
                kA, kB = slice(ks, ks + 64), slice(ks + 64, ks + 128)
                # pair 1: (half A, qc0/bank0) + (half B, qc1/bank1)
                nc.tensor.matmul(pl[0:64, s0], kth[:, kA], qth[:, s0])
                nc.tensor.matmul(pl[64:128, s1], kth[:, kB], qth[:, s1])
                # pair 2: (half B, qc0/bank0) + (half A, qc1/bank1)
                nc.tensor.matmul(pl[64:128, s0], kth[:, kB], qth[:, s0])
                nc.tensor.matmul(pl[0:64, s1], kth[:, kA], qth[:, s1])
                wt = sb_w.tile([128, S], F16, tag="wt", name=f"wt_{h}_{kt}")
                nc.scalar.activation(
                    wt[:], pl[:], mybir.ActivationFunctionType.Exp, scale=SCALE
                )
                wts.append(wt)

            emit_qk(0)
            for kt in range(kt_tiles):
                if kt + 1 < kt_tiles:
                    emit_qk(kt + 1)
                wt = wts[kt]
                ks = kt * 128
                dA, dB = slice(ks, ks + 64), slice(ks + 64, ks + 128)
                first, last = kt == 0, kt == kt_tiles - 1
                ic = slice(kt * 32, kt * 32 + 32)
                order = opts.get("mm_order", "dAA")
                mm_den = [
                    (pd[0:32, s0], ind_sb[:, ic], wt[:, s0]),
                    (pd[32:64, s1], ind_sb[:, ic], wt[:, s1]),
                ]
                mm_av1 = [
                    (po[0:64, s0], vh[:, dA], wt[:, s0]),
                    (po[64:128, s1], vh[:, dB], wt[:, s1]),
                ]
                mm_av2 = [
                    (po[64:128, s0], vh[:, dB], wt[:, s0]),
                    (po[0:64, s1], vh[:, dA], wt[:, s1]),
                ]
                seqs = {"dAA": mm_den + mm_av1 + mm_av2,
                        "AdA": mm_av1 + mm_den + mm_av2,
                        "AAd": mm_av1 + mm_av2 + mm_den}[order]
                for out_ap, w_ap, r_ap in seqs:
                    nc.tensor.matmul(out_ap, w_ap, r_ap, start=first, stop=last)

            # denominator first (releases pd for the next head's den matmuls):
            # rows {0, 32} of pd via a stride-32 partition AP -> [2, 1024]
            dsb = sb_out.tile([33, S], F32, tag="dsb")
            nc.vector.tensor_copy(dsb[:], pd[0:33, :])
            store.dma_start(den_t.ap()[h, :, :], dsb[:])
            # numerator to SBUF, divide on host
            osb = sb_out.tile([128, S], F32, tag="osb")
            copy_op(osb[:], po[:])
            store.dma_start(out_t.ap()[hs:hs + DH, :], osb[:])

    nc.compile()
    return nc


def kernel(memory, query, seq_mask, b):
    memory = np.ascontiguousarray(memory, dtype=np.float32)
    query = np.ascontiguousarray(query, dtype=np.float32)
    seq_mask = np.asarray(seq_mask)
    assert memory.shape == (B, S, 2 * D) and query.shape == (B, S, D)

    counts = [int(np.count_nonzero(seq_mask[i])) for i in range(B)]
    kp = max(max(counts), 1)
    kp = ((kp + 127) // 128) * 128
    kt_tiles = kp // 128

    key = (kt_tiles, tuple(sorted(OPTS.items())))
    if key not in _NC_CACHE:
        _NC_CACHE[key] = _build(kt_tiles, OPTS)
    nc = _NC_CACHE[key]

    q_t = np.ascontiguousarray(query.transpose(0, 2, 1)).astype(np.float16)
    in_maps = []
    for i in range(B):
        idx = np.flatnonzero(seq_mask[i])
        nb = len(idx)
        ktb = np.zeros((D, kp), dtype=np.float16)
        vcb = np.zeros((kp, D), dtype=np.float16)
        indb = np.zeros((kp, 32), dtype=np.float16)
        if nb:
            ktb[:, :nb] = memory[i, idx, :D].T
            vcb[:nb] = memory[i, idx, D:]
            indb[:nb] = 1.0
        in_maps.append({"q_t": q_t[i], "k_t": ktb, "v_c": vcb, "ind": indb})

    res = run_bass_kernel_spmd(nc, in_maps, list(range(B)))
    out = np.empty((B, S, D), dtype=np.float32)
    for i in range(B):
        num = res.results[i]["out_t"]            # [D, S] = [(h d), q]
        dd = res.results[i]["den_t"]             # [H, 33, S]
        den = np.concatenate([dd[:, 0, 0:512], dd[:, 32, 512:1024]], axis=1)
        with np.errstate(divide="ignore", invalid="ignore"):
            out[i] = (num.reshape(H, DH, S) / den[:, None, :]).reshape(D, S).T
        if counts[i] == 0:
            # all keys masked: reference softmax degenerates to uniform
            out[i] = memory[i, :, D:].mean(axis=0)[None, :]
    return out


# revision 30
# speedup vs baseline: 1.0171x; 1.0171x over previous
"""Multi-head attention (B=8, H=8, S=1024, d=128) on 8 TRN2 NeuronCores.

Strategy
--------
- Data-parallel over batch: core i handles batch i (8 cores, B=8).
- Host-side prep (layout only): per batch, compact keys/values to the
  seq_mask-selected rows (zero-padded to a multiple of 128 -> kt_tiles
  k-tiles), pre-transpose Q and compacted K so the contraction dim (d)
  lands on SBUF partitions, and cast matmul operands to fp16 (10-bit
  mantissa, same precision class as the PE's TF32 path, but 2-byte
  weight loads that background-prefetch). An indicator matrix
  ind[k, 32] (1 for real keys) rides along for the softmax denominator.
- Device math per (head h, k-tile kt), all matmuls as column-tiled M=64
  (or M=32) pairs on disjoint PE column groups + disjoint PSUM banks so
  the two streams run concurrently:
    logitsT[k, q]  = K^T[:, kt].T @ Q^T           (PE, contraction d)
    W^T[k, q]      = exp(logitsT * d^-0.5)        (ACT, PSUM -> SBUF fp16)
    outT[d, q]    += V[kt].T   @ W^T              (PE, accum over kt)
    den[q]        += ind[kt].T @ W^T              (PE, accum over kt)
  then copy outT/den to SBUF (DVE) and DMA out; the division happens on
  the host (DVE reciprocal is microcoded and slow).
  The learned scalar bias b cancels in softmax (shift invariance) and
  the -1e30 masking is equivalent to dropping masked keys (exp -> 0),
  which the compaction does exactly.
- Host-side unshard: out[b] = outT.T / den (plus uniform-average
  fallback for a fully-masked batch, where reference degenerates to a
  uniform softmax).
"""
from contextlib import ExitStack

import numpy as np

import concourse.bacc as bacc
import concourse.mybir as mybir
import concourse.tile as tile
from concourse.bass_utils import run_bass_kernel_spmd

F32 = mybir.dt.float32
F32R = mybir.dt.float32r
F16 = mybir.dt.float16

B, S, D, H = 8, 1024, 1024, 8
DH = D // H              # 128, head dim = one partition tile
SCALE = float(DH) ** -0.5
NQC = S // 512           # q chunks of 512 for AV/normalize

_NC_CACHE: dict[tuple, object] = {}

# build options (overridable for profiling experiments)
OPTS: dict = {}


def _build(kt_tiles: int, opts: dict | None = None):
    """Build + compile the per-core kernel for `kt_tiles` 128-wide key tiles."""
    opts = opts or {}
    pl_bufs = opts.get("pl_bufs", 2)
    po_bufs = opts.get("po_bufs", 1)
    copy_eng = opts.get("copy_eng", "vector")   # engine for PSUM->SBUF copies
    store_eng = opts.get("store_eng", "sync")  # engine issuing output DMAs
    KP = kt_tiles * 128
    nc = bacc.Bacc("TRN2", target_bir_lowering=False, debug=False)

    q_t = nc.dram_tensor("q_t", [D, S], F16, kind="ExternalInput")
    k_t = nc.dram_tensor("k_t", [D, KP], F16, kind="ExternalInput")
    v_c = nc.dram_tensor("v_c", [KP, D], F16, kind="ExternalInput")
    ind = nc.dram_tensor("ind", [KP, 32], F16, kind="ExternalInput")
    out_t = nc.dram_tensor("out_t", [D, S], F32, kind="ExternalOutput")
    den_t = nc.dram_tensor("den_t", [H, 33, S], F32, kind="ExternalOutput")

    with tile.TileContext(nc) as tc, ExitStack() as ctx:
        sb_in = ctx.enter_context(tc.tile_pool(name="sb_in", bufs=3))
        sb_ind = ctx.enter_context(tc.tile_pool(name="sb_ind", bufs=1))
        sb_w = ctx.enter_context(tc.tile_pool(name="sb_w", bufs=6))
        sb_out = ctx.enter_context(tc.tile_pool(name="sb_out", bufs=4))
        ps_l = ctx.enter_context(
            tc.tile_pool(name="ps_l", bufs=pl_bufs, space="PSUM"))
        ps_o = ctx.enter_context(
            tc.tile_pool(name="ps_o", bufs=po_bufs, space="PSUM"))
        ps_d = ctx.enter_context(tc.tile_pool(name="ps_d", bufs=1, space="PSUM"))

        store = {"gpsimd": nc.gpsimd, "scalar": nc.scalar, "sync": nc.sync}[store_eng]

        def copy_op(dst, src):
            if copy_eng == "vector":
                nc.vector.tensor_copy(dst, src)
            else:
                nc.scalar.copy(dst, src)

        ind_sb = None

        for h in range(H):
            hs = h * DH
            # first chunks first: the kernel's first matmul needs
            # kth[:, :128] and qth[:, :512] only
            kth = sb_in.tile([128, KP], F16, tag="kth")
            nc.sync.dma_start(kth[:, 0:128], k_t.ap()[hs:hs + DH, 0:128])
            qth = sb_in.tile([128, S], F16, tag="qth")
            nc.sync.dma_start(qth[:, 0:512], q_t.ap()[hs:hs + DH, 0:512])
            if KP > 128:
                nc.sync.dma_start(kth[:, 128:], k_t.ap()[hs:hs + DH, 128:])
            nc.sync.dma_start(qth[:, 512:], q_t.ap()[hs:hs + DH, 512:])
            if ind_sb is None:
                # indicator tiles: [128(k), 32] per k-tile, concatenated on
                # the free dim. 32 columns keeps their LDWEIGHTS cheap.
                ind_sb = sb_ind.tile([128, kt_tiles * 32], F16)
                nc.sync.dma_start(
                    ind_sb[:].rearrange("p (t c) -> p t c", c=32),
                    ind.ap().rearrange("(t p) c -> p t c", p=128),
                )
            # V for this head: [128(k), 128(d)] tiles concatenated on free dim
            vh = sb_in.tile([128, KP], F16, tag="vh")
            nc.sync.dma_start(
                vh[:].rearrange("p (t c) -> p t c", c=DH),
                v_c.ap()[:, hs:hs + DH].rearrange("(t p) c -> p t c", p=128),
            )

            po = ps_o.tile([128, S], F32, tag="po")    # outT accum [d, q]
            # denominator: [0:32, 0:512] = q-chunk 0, [32:64, 512:] = q-chunk 1
            pd = ps_d.tile([64, S], F32, tag="pd")

            # Column-tiled matmul pairs: two M=64 matmuls on disjoint PE
            # column groups AND disjoint PSUM banks stream concurrently.
            # Pair diagonally across (column-half, q-chunk) so banks differ.
            s0, s1 = slice(0, 512), slice(512, 1024)
            wts = []

            def emit_qk(kt):
                pl = ps_l.tile([128, S], F32, tag="pl", name=f"pl_{h}_{kt}")
                ks = kt * 128
                kA, kB = slice(ks, ks + 64), slice(ks + 64, ks + 128)
                # pair 1: (half A, qc0/bank0) + (half B, qc1/bank1)
                nc.tensor.matmul(pl[0:64, s0], kth[:, kA], qth[:, s0])
                nc.tensor.matmul(pl[64:128, s1], kth[:, kB], qth[:, s1])
                # pair 2: (half B, qc0/bank0) + (half A, qc1/bank1)
                nc.tensor.matmul(pl[64:128, s0], kth[:, kB], qth[:, s0])
                nc.tensor.matmul(pl[0:64, s1], kth[:, kA], qth[:, s1])
                wt = sb_w.tile([128, S], F16, tag="wt", name=f"wt_{h}_{kt}")
                nc.scalar.activation(
                    wt[:], pl[:], mybir.ActivationFunctionType.Exp, scale=SCALE
                )
                wts.append(wt)

            emit_qk(0)
            for kt in range(kt_tiles):
                if kt + 1 < kt_tiles:
                    emit_qk(kt + 1)
                wt = wts[kt]
                ks = kt * 128
                dA, dB = slice(ks, ks + 64), slice(ks + 64, ks + 128)
                first, last = kt == 0, kt == kt_tiles - 1
                ic = slice(kt * 32, kt * 32 + 32)
                order = opts.get("mm_order", "dAA")
                mm_den = [
                    (pd[0:32, s0], ind_sb[:, ic], wt[:, s0]),
                    (pd[32:64, s1], ind_sb[:, ic], wt[:, s1]),
                ]
                mm_av1 = [
                    (po[0:64, s0], vh[:, dA], wt[:, s0]),
                    (po[64:128, s1], vh[:, dB], wt[:, s1]),
                ]
                mm_av2 = [
                    (po[64:128, s0], vh[:, dB], wt[:, s0]),
                    (po[0:64, s1], vh[:, dA], wt[:, s1]),
                ]
                seqs = {"dAA": mm_den + mm_av1 + mm_av2,
                        "AdA": mm_av1 + mm_den + mm_av2,
                        "AAd": mm_av1 + mm_av2 + mm_den}[order]
                for out_ap, w_ap, r_ap in seqs:
                    nc.tensor.matmul(out_ap, w_ap, r_ap, start=first, stop=last)

            # denominator first (releases pd for the next head's den matmuls);
            # rows 0 and 32 carry the real values, host picks them out
            dsb = sb_out.tile([33, S], F32, tag="dsb")
            nc.vector.tensor_copy(dsb[:], pd[0:33, :])
            store.dma_start(den_t.ap()[h, :, :], dsb[:])
            # numerator to SBUF, divide on host
            osb = sb_out.tile([128, S], F32, tag="osb")
            copy_op(osb[:], po[:])
            store.dma_start(out_t.ap()[hs:hs + DH, :], osb[:])

    nc.compile()
    return nc


def kernel(memory, query, seq_mask, b):
    memory = np.ascontiguousarray(memory, dtype=np.float32)
    query = np.ascontiguousarray(query, dtype=np.float32)
    seq_mask = np.asarray(seq_mask)
    assert memory.shape == (B, S, 2 * D) and query.shape == (B, S, D)

    counts = [int(np.count_nonzero(seq_mask[i])) for i in range(B)]
    kp = max(max(counts), 1)
    kp = ((kp + 127) // 128) * 128
    kt_tiles = kp // 128

    key = (kt_tiles, tuple(sorted(OPTS.items())))
    if key not in _NC_CACHE:
        _NC_CACHE[key] = _build(kt_tiles, OPTS)
    nc = _NC_CACHE[key]

    q_t = np.ascontiguousarray(query.transpose(0, 2, 1)).astype(np.float16)
    in_maps = []
    for i in range(B):
        idx = np.flatnonzero(seq_mask[i])
        nb = len(idx)
        ktb = np.zeros((D, kp), dtype=np.float16)
        vcb = np.zeros((kp, D), dtype=np.float16)
        indb = np.zeros((kp, 32), dtype=np.float16)
        if nb:
            ktb[:, :nb] = memory[i, idx, :D].T
            vcb[:nb] = memory[i, idx, D:]
            indb[:nb] = 1.0
        in_maps.append({"q_t": q_t[i], "k_t": ktb, "v_c": vcb, "ind": indb})

    res = run_bass_kernel_spmd(nc, in_maps, list(range(B)))
    out = np.empty((B, S, D), dtype=np.float32)
    for i in range(B):
        num = res.results[i]["out_t"]            # [D, S] = [(h d), q]
        dd = res.results[i]["den_t"]             # [H, 33, S]
        den = np.concatenate([dd[:, 0, 0:512], dd[:, 32, 512:1024]], axis=1)
        with np.errstate(divide="ignore", invalid="ignore"):
            out[i] = (num.reshape(H, DH, S) / den[:, None, :]).reshape(D, S).T
        if counts[i] == 0:
            # all keys masked: reference softmax degenerates to uniform
            out[i] = memory[i, :, D:].mean(axis=0)[None, :]
    return out
